# revision 1
# baseline (speedup 1.0000x reference)
"""CoAttenBlock Trainium2 kernel.

Full inputs in, full outputs out. Shards batch (B=8) across 8 NeuronCores,
one sample per core (pure data parallel, no collectives).

Per-core math (C=64, HW=2304, strips of 128 along the left position axis n):
  XL = WL @ [xlh;xll] + bL                      [64, 2304]
  XR = WR @ [xrh;xrl] + bR
  per strip s: aff_s = XL[:,s].T @ XR           [128, 2304]
               E_s   = exp(aff_s), rowsum via activation accum_out
               r2    = 1/rowsum  (folded into the strip's YRT weight columns)
               P12  += [YLT_s | YRT_s*r2].T @ E_s   (PSUM-resident [128, 2304])
  where YLT = (WLo_r @ XL).T strips, YRT = (WRo_r @ XR).T strips, so that
  P1 = WLo_r @ (XL @ E) and P2 = WRo_r @ (XR @ diag(r2) @ E).
  Gate pre-acts are recovered as vL.T @ P1 with vL = solve(WLo_r.T, gwL.T)
  (host-side 64x64 solve; inputs are deterministic, conditioning ~5e2).
  colsum = ones.T @ (sum of E_s)  (two SBUF accumulators: DVE + GPSIMD chains)
  s1 = sigmoid(g1pre * r1 + gb1) * r1,  r1 = 1/colsum ; s2 = sigmoid(g2pre+gb2)
  out_L = WLo_l @ XL + P1 * s1[m] + bLo ; out_R = WRo_l @ XR + P2 * s2[m] + bRo

float32r (single-pass PE mode) is used for all large matmuls; producers of
f32r-matmul inputs write with a f32r-typed output AP so the engine rounds on
write (BIR verifier requirement). Convs and YLT/YRT stay higher precision.
"""

import os
import sys

import numpy as np

if os.path.isdir("/opt/trn_rl_repo") and "/opt/trn_rl_repo" not in sys.path:
    sys.path.insert(0, "/opt/trn_rl_repo")

import concourse.bass as bass
import concourse.tile as tile
from concourse import bacc, mybir
from concourse.bass_utils import run_bass_kernel_spmd

B, C, H, W = 8, 64, 48, 48
HW = H * W            # 2304
C2 = 2 * C            # 128
NSTRIP = HW // 128    # 18
F32 = mybir.dt.float32
F32R = mybir.dt.float32r
AF = mybir.ActivationFunctionType


def chunks(total, step=512):
    out = []
    c0 = 0
    while c0 < total:
        out.append((c0, min(step, total - c0)))
        c0 += step
    return out


CH_2304 = chunks(2304)            # 4x512 + 256


def r(ap):
    return ap.bitcast(F32R)


def build_nc():
    nc = bacc.Bacc("TRN2", target_bir_lowering=False, debug=False)

    x2l_d = nc.dram_tensor("x2l", [C2, HW], F32, kind="ExternalInput").ap()
    x2r_d = nc.dram_tensor("x2r", [C2, HW], F32, kind="ExternalInput").ap()
    wlT_d = nc.dram_tensor("wlT", [C2, C], F32, kind="ExternalInput").ap()
    wrT_d = nc.dram_tensor("wrT", [C2, C], F32, kind="ExternalInput").ap()
    wloRT_d = nc.dram_tensor("wloRT", [C, C], F32, kind="ExternalInput").ap()
    wroRT_d = nc.dram_tensor("wroRT", [C, C], F32, kind="ExternalInput").ap()
    wloLT_d = nc.dram_tensor("wloLT", [C, C], F32, kind="ExternalInput").ap()
    wroLT_d = nc.dram_tensor("wroLT", [C, C], F32, kind="ExternalInput").ap()
    vlr_d = nc.dram_tensor("vlr", [C2, 1], F32, kind="ExternalInput").ap()
    bL_d = nc.dram_tensor("bL", [C, 1], F32, kind="ExternalInput").ap()
    bR_d = nc.dram_tensor("bR", [C, 1], F32, kind="ExternalInput").ap()
    bLo_d = nc.dram_tensor("bLo", [C, 1], F32, kind="ExternalInput").ap()
    bRo_d = nc.dram_tensor("bRo", [C, 1], F32, kind="ExternalInput").ap()
    gb_d = nc.dram_tensor("gb", [1, 2], F32, kind="ExternalInput").ap()
    # identity stacked twice: rows 0:64 and 64:128 both hold I_64, so id-adds
    # can source either half of a [128, ...] tile at matching base partition
    id2_np = np.vstack([np.eye(C, dtype=np.float32), np.eye(C, dtype=np.float32)])
    id64b_d = nc.inline_tensor(id2_np, "id64b").ap()
    # selector for the merged S12 broadcast: out rows 0:64 <- s1, 64:128 <- g2
    sel_np = np.zeros((2, C2), dtype=np.float32)
    sel_np[0, 0:C] = 1.0
    sel_np[1, C:C2] = 1.0
    sel12_d = nc.inline_tensor(sel_np, "sel12").ap()

    out_l_d = nc.dram_tensor("out_l", [C, HW], F32, kind="ExternalOutput").ap()
    out_r_d = nc.dram_tensor("out_r", [C, HW], F32, kind="ExternalOutput").ap()

    with tile.TileContext(nc) as tc:
        import contextlib

        with contextlib.ExitStack() as outer:
            consts = outer.enter_context(tc.tile_pool(name="consts", bufs=1))
            big = outer.enter_context(tc.tile_pool(name="big", bufs=1))
            epool = outer.enter_context(tc.tile_pool(name="epool", bufs=4))
            smalls = outer.enter_context(tc.tile_pool(name="smalls", bufs=3))
            ph3sb = outer.enter_context(tc.tile_pool(name="ph3sb", bufs=2))

            # ---- constants / weights to SBUF ----
            wlT = consts.tile([C2, C], F32)
            wrT = consts.tile([C2, C], F32)
            wloRT_raw = consts.tile([C, C], F32)
            wroRT_raw = consts.tile([C, C], F32)
            wloLT_raw = consts.tile([C, C], F32)
            wroLT_raw = consts.tile([C, C], F32)
            vlr_raw = consts.tile([C2, 1], F32)
            id64b_raw = consts.tile([C2, C], F32)
            sel12_raw = consts.tile([2, C2], F32)
            ones128_raw = consts.tile([C2, 1], F32)
            ones64_raw = consts.tile([1, C], F32)
            wloRT = consts.tile([C, C], F32)
            wroRT = consts.tile([C, C], F32)
            wloLT = consts.tile([C, C], F32)
            wroLT = consts.tile([C, C], F32)
            vlr = consts.tile([C2, 1], F32)
            id64b = consts.tile([C2, C], F32)
            sel12 = consts.tile([2, C2], F32)
            ones128 = consts.tile([C2, 1], F32)
            ones64 = consts.tile([1, C], F32)
            bL = consts.tile([C, 1], F32)
            bR = consts.tile([C, 1], F32)
            bLo = consts.tile([C, 1], F32)
            bRo = consts.tile([C, 1], F32)
            gb = consts.tile([1, 2], F32)
            for dst, src in [
                (r(wlT), r(wlT_d)), (r(wrT), r(wrT_d)),
                (wloRT_raw, wloRT_d), (wroRT_raw, wroRT_d),
                (wloLT_raw, wloLT_d), (wroLT_raw, wroLT_d),
                (vlr_raw, vlr_d), (id64b_raw, id64b_d), (sel12_raw, sel12_d),
                (bL, bL_d), (bR, bR_d), (bLo, bLo_d), (bRo, bRo_d),
                (gb, gb_d),
            ]:
                nc.sync.dma_start(out=dst, in_=src)
            nc.vector.memset(ones128_raw, 1.0)
            nc.vector.memset(ones64_raw, 1.0)
            for dst, srcc in [(ones128, ones128_raw), (ones64, ones64_raw),
                              (wloRT, wloRT_raw), (wroRT, wroRT_raw),
                              (wloLT, wloLT_raw), (wroLT, wroLT_raw),
                              (vlr, vlr_raw), (id64b, id64b_raw),
                              (sel12, sel12_raw)]:
                nc.scalar.copy(r(dst), srcc)

            # ---- big SBUF tensors ----
            x2l = big.tile([C2, HW], F32)
            x2r = big.tile([C2, HW], F32)
            XL = big.tile([C, HW], F32)
            XR = big.tile([C, HW], F32)
            Wc = big.tile([C2, HW], F32)       # 18 strips of [YLT | YRT]
            csum_a = big.tile([C2, HW // 2], F32)  # DVE accumulates m[0:1152]
            csum_b = big.tile([C2, HW // 2], F32)  # Pool accumulates m[1152:]
            P12sb = big.tile([C2, HW], F32)    # drained P1 (0:64) / P2 (64:128)
            outLR = big.tile([C2, HW], F32)

            for c0, cn in CH_2304:
                nc.sync.dma_start(out=r(x2l[:, c0:c0 + cn]),
                                  in_=r(x2l_d[:, c0:c0 + cn]))
                nc.sync.dma_start(out=r(x2r[:, c0:c0 + cn]),
                                  in_=r(x2r_d[:, c0:c0 + cn]))

            with contextlib.ExitStack() as ph2_psum:
                p12p = ph2_psum.enter_context(
                    tc.tile_pool(name="p12p", bufs=1, space="PSUM"))
                affp = ph2_psum.enter_context(
                    tc.tile_pool(name="affp", bufs=1, space="PSUM"))
                P12 = p12p.tile([C2, HW], F32)  # 5 banks, lives all of phase 1+2
                ring = affp.tile([C2, 1536], F32, tag="ring", name="aff_ring")

                # ---- phase 1: convs (full fp32) + YLT/YRT build ----
                # After conv chunk j, emit the YLT/YRT strips of chunk j-1
                # (their XL/XR columns are copied by then); P12 is scratch.
                def emit_y(t):
                    ysl = slice(64 * t, 64 * t + 64)
                    nc.tensor.matmul(P12[:, ysl],
                                     r(XL[:, 128 * t:128 * t + 128]),
                                     r(wloRT), start=True, stop=True)
                    nc.vector.tensor_copy(r(Wc[:, 128 * t:128 * t + 64]),
                                          P12[:, ysl])
                    ysr = slice(64 * (NSTRIP + t), 64 * (NSTRIP + t) + 64)
                    nc.tensor.matmul(P12[:, ysr],
                                     r(XR[:, 128 * t:128 * t + 128]),
                                     r(wroRT), start=True, stop=True)
                    nc.vector.tensor_copy(r(Wc[:, 128 * t + 64:128 * t + 128]),
                                          P12[:, ysr])

                for j, (c0, cn) in enumerate(CH_2304):
                    nc.tensor.matmul(P12[0:C, c0:c0 + cn], r(wlT),
                                     r(x2l[:, c0:c0 + cn]), start=True, stop=True)
                    nc.scalar.activation(r(XL[:, c0:c0 + cn]),
                                         P12[0:C, c0:c0 + cn],
                                         AF.Identity, bias=bL, scale=1.0)
                    rsl = (j % 3) * 512
                    nc.tensor.matmul(ring[0:C, rsl:rsl + cn], r(wrT),
                                     r(x2r[:, c0:c0 + cn]), start=True, stop=True)
                    nc.vector.tensor_scalar_add(r(XR[:, c0:c0 + cn]),
                                                ring[0:C, rsl:rsl + cn], bR)
                    if j > 0:
                        for t in range(4 * (j - 1), 4 * j):
                            emit_y(t)
                for t in range(4 * (len(CH_2304) - 1), NSTRIP):
                    emit_y(t)

                # ---- phase 2: strip loop over a 3-slot aff ring ----
                # A_s = aff matmuls + merged exps + rowsum/recip for strip s.
                # B_s = Wc scale + bacc matmuls + colsum accumulate for s.
                # B lags A by 2 strips so PE always has bacc work to fill exp
                # waits; the YLT/YRT -> Wc build is emitted during the lag.
                phase = 0
                r2s = {}

                def emit_bacc(sb, c0, cn):
                    nc.tensor.matmul(P12[:, c0:c0 + cn],
                                     r(Wc[:, 128 * sb:128 * sb + 128]),
                                     r(Es[sb][:, c0:c0 + cn]),
                                     start=(sb == 0), stop=(sb == NSTRIP - 1))

                def emit_csum(sb):
                    E = Es[sb]
                    half = HW // 2
                    if sb == 0:
                        nc.vector.tensor_copy(r(csum_a), E[:, 0:half])
                        nc.gpsimd.tensor_copy(r(csum_b), E[:, half:HW])
                    else:
                        nc.vector.tensor_add(r(csum_a), csum_a, E[:, 0:half])
                        nc.gpsimd.tensor_add(r(csum_b), csum_b, E[:, half:HW])

                def emit_A(s, phase, sb):
                    # aff+exp for strip s, with strip sb's bacc matmuls
                    # interleaved between the aff pieces (PE is in-order; this
                    # keeps ACT fed with the next exp as early as possible).
                    if sb >= 0:
                        wright = Wc[:, 128 * sb + 64:128 * sb + 128]
                        nc.vector.tensor_scalar_mul(r(wright), wright, r2s[sb])
                    E = epool.tile([C2, HW], F32, tag="e", name=f"E_{s}")
                    rs = smalls.tile([C2, 4], F32, tag="rs", name=f"rs_{s}")
                    lhs_aff = r(XL[:, 128 * s:128 * s + 128])
                    pieces = [(p0, pn, (phase + i) % 3)
                              for i, (p0, pn) in enumerate(CH_2304)]
                    groups = []
                    for p0, pn, sl in pieces:
                        if groups and groups[-1][2] + groups[-1][1] == sl * 512 \
                                and groups[-1][1] + pn <= 1536:
                            groups[-1][1] += pn
                        else:
                            groups.append([p0, pn, sl * 512])
                    gidx = 0
                    done = 0
                    for i, (p0, pn, sl) in enumerate(pieces):
                        nc.tensor.matmul(ring[:, sl * 512:sl * 512 + pn],
                                         lhs_aff, r(XR[:, p0:p0 + pn]),
                                         start=True, stop=True)
                        done += pn
                        while gidx < len(groups) and \
                                groups[gidx][0] + groups[gidx][1] <= done:
                            m0, mn, r0 = groups[gidx]
                            nc.scalar.activation(r(E[:, m0:m0 + mn]),
                                                 ring[:, r0:r0 + mn], AF.Exp,
                                                 accum_out=rs[:, gidx:gidx + 1])
                            gidx += 1
                        if sb >= 0 and i < len(CH_2304):
                            bc0, bcn = CH_2304[i]
                            emit_bacc(sb, bc0, bcn)
                    rowsum = smalls.tile([C2, 1], F32, tag="rowsum",
                                         name=f"rowsum_{s}")
                    r2 = smalls.tile([C2, 1], F32, tag="r2", name=f"r2_{s}",
                                     bufs=4)
                    nc.vector.tensor_reduce(rowsum, rs[:, 0:len(groups)],
                                            axis=mybir.AxisListType.X,
                                            op=mybir.AluOpType.add)
                    nc.vector.reciprocal(r2, rowsum)
                    r2s[s] = r2
                    if sb >= 0:
                        emit_csum(sb)
                    return E

                def emit_B_tail(sb):
                    wright = Wc[:, 128 * sb + 64:128 * sb + 128]
                    nc.vector.tensor_scalar_mul(r(wright), wright, r2s[sb])
                    for c0, cn in CH_2304:
                        emit_bacc(sb, c0, cn)
                    emit_csum(sb)

                Es = {}
                Es = {}

                for s in range(NSTRIP):
                    Es[s] = emit_A(s, phase, s - 2)
                    phase = (phase + len(CH_2304)) % 3
                for s in (NSTRIP - 2, NSTRIP - 1):
                    emit_B_tail(s)

                # drain P12 (both engines in parallel)
                nc.vector.tensor_copy(r(P12sb[0:C, :]), P12[0:C, :])
                nc.scalar.copy(r(P12sb[C:C2, :]), P12[C:C2, :])

            # ---- phase 3: 512-col pieces, one PSUM bank per role ----
            with tc.tile_pool(name="ph3p", bufs=1, space="PSUM") as ph3:
                for q, (p0, pn) in enumerate(CH_2304):
                    sl = slice(p0, p0 + pn)

                    cs = ph3.tile([1, pn], F32, tag="cs", name=f"cs_{q}",
                                  padded_shape=[1, 512])
                    half = HW // 2
                    if p0 + pn <= half:
                        nc.tensor.matmul(cs, r(ones128),
                                         r(csum_a[:, p0:p0 + pn]),
                                         start=True, stop=True)
                    elif p0 >= half:
                        nc.tensor.matmul(cs, r(ones128),
                                         r(csum_b[:, p0 - half:p0 - half + pn]),
                                         start=True, stop=True)
                    else:
                        ca = half - p0
                        nc.tensor.matmul(cs[:, 0:ca], r(ones128),
                                         r(csum_a[:, p0:half]),
                                         start=True, stop=True)
                        nc.tensor.matmul(cs[:, ca:pn], r(ones128),
                                         r(csum_b[:, 0:p0 + pn - half]),
                                         start=True, stop=True)
                    r1 = ph3sb.tile([1, pn], F32, tag="r1", name=f"r1_{q}",
                                    padded_shape=[1, 512])
                    nc.vector.reciprocal(r1, cs)

                    g1p = ph3.tile([1, pn], F32, tag="g1p", name=f"g1p_{q}",
                                   padded_shape=[1, 512])
                    nc.tensor.matmul(g1p, r(vlr[0:C]), r(P12sb[0:C, sl]),
                                     start=True, stop=True)
                    g2p = ph3.tile([1, pn], F32, tag="g2p", name=f"g2p_{q}",
                                   padded_shape=[1, 512])
                    nc.tensor.matmul(g2p, r(vlr[C:C2]), r(P12sb[C:C2, sl]),
                                     start=True, stop=True)

                    g1pre = ph3sb.tile([1, pn], F32, tag="g1pre",
                                       name=f"g1pre_{q}", padded_shape=[1, 512])
                    nc.vector.tensor_mul(g1pre, g1p, r1)
                    g1 = ph3sb.tile([1, pn], F32, tag="g1", name=f"g1_{q}",
                                    padded_shape=[1, 512])
                    nc.scalar.activation(g1, g1pre, AF.Sigmoid,
                                         bias=gb[0:1, 0:1], scale=1.0)
                    s1 = ph3sb.tile([1, pn], F32, tag="s1", name=f"s1_{q}",
                                    padded_shape=[1, 512])
                    nc.vector.tensor_mul(r(s1), g1, r1)
                    g2 = ph3sb.tile([1, pn], F32, tag="g2", name=f"g2_{q}",
                                    padded_shape=[1, 512])
                    nc.scalar.activation(r(g2), g2p, AF.Sigmoid,
                                         bias=gb[0:1, 1:2], scale=1.0)

                    S1 = ph3.tile([C, pn], F32, tag="S1", name=f"S1_{q}",
                                  padded_shape=[C, 512])
                    nc.tensor.matmul(S1, r(ones64), r(s1), start=True, stop=True)
                    S2 = ph3.tile([C, pn], F32, tag="S2", name=f"S2_{q}",
                                  padded_shape=[C, 512])
                    nc.tensor.matmul(S2, r(ones64), r(g2), start=True, stop=True)
                    t1 = ph3sb.tile([C, pn], F32, tag="t1", name=f"t1_{q}",
                                    padded_shape=[C, 512])
                    nc.vector.tensor_mul(r(t1), P12sb[0:C, sl], S1)
                    t2 = ph3sb.tile([C, pn], F32, tag="t2", name=f"t2_{q}",
                                    padded_shape=[C, 512])
                    nc.vector.tensor_mul(r(t2), P12sb[C:C2, sl], S2)

                    OL = ph3.tile([C, pn], F32, tag="OL", name=f"OL_{q}",
                                  padded_shape=[C, 512])
                    nc.tensor.matmul(OL, r(wloLT), r(XL[:, sl]),
                                     start=True, stop=False)
                    nc.tensor.matmul(OL, r(id64b[0:C]), r(t1),
                                     start=False, stop=True)
                    nc.scalar.activation(outLR[0:C, sl], OL, AF.Identity,
                                         bias=bLo, scale=1.0)
                    OR_ = ph3.tile([C, pn], F32, tag="OR", name=f"OR_{q}",
                                   padded_shape=[C, 512])
                    nc.tensor.matmul(OR_, r(wroLT), r(XR[:, sl]),
                                     start=True, stop=False)
                    nc.tensor.matmul(OR_, r(id64b[0:C]), r(t2),
                                     start=False, stop=True)
                    nc.scalar.activation(outLR[C:C2, sl], OR_, AF.Identity,
                                         bias=bRo, scale=1.0)
                    nc.sync.dma_start(out=out_l_d[:, sl], in_=outLR[0:C, sl])
                    nc.sync.dma_start(out=out_r_d[:, sl], in_=outLR[C:C2, sl])

    nc.compile()
    return nc


_NC_CACHE = {}


def _get_nc():
    if "nc" not in _NC_CACHE:
        _NC_CACHE["nc"] = build_nc()
    return _NC_CACHE["nc"]


def _prep_shared(concaL_w, concaL_b, concaR_w, concaR_b,
                 gateL_w, gateL_b, gateR_w, gateR_b,
                 concaLo_w, concaLo_b, concaRo_w, concaRo_b):
    f = np.float32
    wloR = np.asarray(concaLo_w)[:, C:].astype(np.float64)
    wroR = np.asarray(concaRo_w)[:, C:].astype(np.float64)
    vL = np.linalg.solve(wloR.T, np.asarray(gateL_w).astype(np.float64).reshape(C))
    vR = np.linalg.solve(wroR.T, np.asarray(gateR_w).astype(np.float64).reshape(C))
    vlr = np.concatenate([vL, vR]).reshape(C2, 1)
    return {
        "wlT": np.ascontiguousarray(np.asarray(concaL_w).T, dtype=f),
        "wrT": np.ascontiguousarray(np.asarray(concaR_w).T, dtype=f),
        "wloRT": np.ascontiguousarray(wloR.T, dtype=f),
        "wroRT": np.ascontiguousarray(wroR.T, dtype=f),
        "wloLT": np.ascontiguousarray(np.asarray(concaLo_w)[:, :C].T, dtype=f),
        "wroLT": np.ascontiguousarray(np.asarray(concaRo_w)[:, :C].T, dtype=f),
        "vlr": np.ascontiguousarray(vlr, dtype=f),
        "bL": np.ascontiguousarray(np.asarray(concaL_b).reshape(C, 1), dtype=f),
        "bR": np.ascontiguousarray(np.asarray(concaR_b).reshape(C, 1), dtype=f),
        "bLo": np.ascontiguousarray(np.asarray(concaLo_b).reshape(C, 1), dtype=f),
        "bRo": np.ascontiguousarray(np.asarray(concaRo_b).reshape(C, 1), dtype=f),
        "gb": np.array([[np.asarray(gateL_b).reshape(()),
                         np.asarray(gateR_b).reshape(())]], dtype=f),
    }


def kernel(xlh, xll, xrh, xrl,
           concaL_w, concaL_b, concaR_w, concaR_b,
           gateL_w, gateL_b, gateR_w, gateR_b,
           concaLo_w, concaLo_b, concaRo_w, concaRo_b,
           _return_results=False):
    nc = _get_nc()
    shared = _prep_shared(concaL_w, concaL_b, concaR_w, concaR_b,
                          gateL_w, gateL_b, gateR_w, gateR_b,
                          concaLo_w, concaLo_b, concaRo_w, concaRo_b)
    xlh = np.asarray(xlh, dtype=np.float32)
    xll = np.asarray(xll, dtype=np.float32)
    xrh = np.asarray(xrh, dtype=np.float32)
    xrl = np.asarray(xrl, dtype=np.float32)

    in_maps = []
    for c in range(B):
        x2l = np.concatenate([xlh[c].reshape(C, HW), xll[c].reshape(C, HW)], axis=0)
        x2r = np.concatenate([xrh[c].reshape(C, HW), xrl[c].reshape(C, HW)], axis=0)
        m = dict(shared)
        m["x2l"] = np.ascontiguousarray(x2l)
        m["x2r"] = np.ascontiguousarray(x2r)
        in_maps.append(m)

    # The first execution of a freshly compiled NEFF occasionally hits a
    # transient NRT_EXEC_UNIT_UNRECOVERABLE on this axon setup; an immediate
    # re-dispatch of the same executable has always succeeded, so retry.
    res = None
    for attempt in range(3):
        try:
            res = run_bass_kernel_spmd(nc, in_maps, list(range(B)))
            break
        except Exception:
            if attempt == 2:
                raise
            import time as _time
            _time.sleep(2.0)
    out_L = np.stack([res.results[c]["out_l"].reshape(C, H, W) for c in range(B)])
    out_R = np.stack([res.results[c]["out_r"].reshape(C, H, W) for c in range(B)])
    if _return_results:
        return (out_L, out_R), res
    return (out_L, out_R)



# revision 15
# speedup vs baseline: 1.1835x; 1.1835x over previous
"""CoAttenBlock Trainium2 kernel (v2: ACT-bound pipeline).

Full inputs in, full outputs out. Shards batch (B=8) across 8 NeuronCores,
one sample per core (pure data parallel, no collectives).

Per-core math (C=64, HW=2304, 18 strips of 128 along the left position n):
  XL = WL @ [xlh;xll] + bL                      [64, 2304]
  XR = WR @ [xrh;xrl] + bR
  per strip s: aff_s = XL[:,s].T @ XR           [128, 2304] (PSUM ring)
               E_s   = exp(aff_s) -> bf16 SBUF (all 18 strips resident),
                       rowsum via activation accum_out
               r2    = 1/rowsum folded into the strip's YRT weight columns
               P12  += [YLT_s | YRT_s*r2].T @ E_s   (PSUM [128, 2304])
  colsum accumulated as csum_a (DVE, cols 0:1536) + csum_b (Pool, 1536:).
  Phase 3 runs the gate math transposed ([128, 18] layout, m = 128*q + p):
    colsumT[p,q] via per-block matmuls csum_block.T @ ones
    g1pT/g2pT via per-block matmuls P12sb_block.T @ v  (v = solve(W_r.T, gw))
    s1 = sigmoid(g1p*r1 + gb1)*r1, s2 = sigmoid(g2p + gb2)  (all [128,18])
    S12[c, 128q+p] = s12T[p, q(c)] via broadcast-lhsT matmuls vs identity
    out = W_l @ X + I @ (P12sb * S12) + b  (accumulated in PSUM, bias on ACT)

E/Wc are bf16 (keeps 18 E strips in SBUF and speeds DVE); everything on the
rowsum/colsum/P12 accumulation paths stays f32. float32r single-pass mode for
the f32 matmuls; producers of f32r matmul inputs write through f32r-typed APs.
"""

import os
import sys

import numpy as np

if os.path.isdir("/opt/trn_rl_repo") and "/opt/trn_rl_repo" not in sys.path:
    sys.path.insert(0, "/opt/trn_rl_repo")

import concourse.bass as bass
import concourse.tile as tile
from concourse import bacc, mybir
from concourse.bass_utils import run_bass_kernel_spmd

B, C, H, W = 8, 64, 48, 48
HW = H * W            # 2304
C2 = 2 * C            # 128
NSTRIP = HW // 128    # 18
F32 = mybir.dt.float32
F32R = mybir.dt.float32r
BF16 = mybir.dt.bfloat16
AF = mybir.ActivationFunctionType

# cpack column map (single staged const tensor [128, NCPACK] f32)
CP_WLORT = 0      # [0:64, 0:64]    wloRT
CP_WRORT = 64     # [0:64, 64:128]  wroRT
CP_WLOLT = 128    # [0:64, 128:192] wloLT
CP_WROLT = 192    # [0:64, 192:256] wroLT
CP_VLR = 256      # [128, 1]
CP_BLR = 257      # bL (0:64) | bR (64:128)
CP_BLRO = 258     # bLo | bRo
CP_GB1 = 259      # gateL_b replicated
CP_GB2 = 260      # gateR_b replicated
CP_ID64B = 261    # [128, 64] I64 stacked twice
CP_ID128 = 325    # [128, 128] I128
CP_BRO0 = 453     # bRo at partitions 0:64 (matmul dst must be base-0)
NCPACK = 454


def chunks(total, step=512):
    out = []
    c0 = 0
    while c0 < total:
        out.append((c0, min(step, total - c0)))
        c0 += step
    return out


CH_2304 = chunks(2304)            # 4x512 + 256
CSPLIT = 1152                     # csum_a (DVE) cols [0:1152), csum_b rest


def r(ap):
    return ap.bitcast(F32R)


def build_nc():
    nc = bacc.Bacc("TRN2", target_bir_lowering=False, debug=False)

    x2l_d = nc.dram_tensor("x2l", [C2, HW], F32, kind="ExternalInput").ap()
    x2r_d = nc.dram_tensor("x2r", [C2, HW], F32, kind="ExternalInput").ap()
    wlrT_d = nc.dram_tensor("wlrT", [C2, C2], F32, kind="ExternalInput").ap()
    cpack_d = nc.dram_tensor("cpack", [C2, NCPACK], F32,
                             kind="ExternalInput").ap()
    selpack_d = nc.dram_tensor("selpack", [2 * NSTRIP, HW], BF16,
                               kind="ExternalInput").ap()

    out_l_d = nc.dram_tensor("out_l", [C, HW], F32, kind="ExternalOutput").ap()
    out_r_d = nc.dram_tensor("out_r", [C, HW], F32, kind="ExternalOutput").ap()

    with tile.TileContext(nc) as tc:
        import contextlib

        with contextlib.ExitStack() as outer:
            consts = outer.enter_context(tc.tile_pool(name="consts", bufs=1))
            big = outer.enter_context(tc.tile_pool(name="big", bufs=1))
            epool = outer.enter_context(tc.tile_pool(name="epool", bufs=NSTRIP))
            smalls = outer.enter_context(tc.tile_pool(name="smalls", bufs=3))
            ph3sb = outer.enter_context(tc.tile_pool(name="ph3sb", bufs=2))

            # ---- big SBUF tensors ----
            x2l = big.tile([C2, HW], F32)
            x2r = big.tile([C2, HW], F32)
            XL = big.tile([C, HW], F32)
            XR = big.tile([C, HW], F32)
            Wc = big.tile([C2, HW], BF16)      # 18 strips of [YLT | YRT]
            csum_a = big.tile([C2, CSPLIT], F32)       # DVE accumulator
            csum_b = big.tile([C2, HW - CSPLIT], F32)  # Pool accumulator
            P12sb = big.tile([C2, HW], F32)    # drained P1 (0:64) / P2 (64:128)
            outLR = big.tile([C2, HW], F32)

            # ---- constants / weights ----
            wlrT = consts.tile([C2, C2], F32)
            cpack = consts.tile([C2, NCPACK], F32)
            selpack = consts.tile([2 * NSTRIP, HW], BF16)
            id128b = consts.tile([C2, C2], BF16)
            id64b = consts.tile([C2, C], BF16)
            ones128 = consts.tile([C2, 1], F32)

            # inputs first (convs are the critical path), split across the
            # two HWDGE queues (SP + ACT) so descriptor-gen overlaps
            nc.sync.dma_start(out=r(wlrT), in_=r(wlrT_d))
            for j, (c0, cn) in enumerate(CH_2304):
                nc.scalar.dma_start(out=r(x2r[:, c0:c0 + cn]),
                                    in_=r(x2r_d[:, c0:c0 + cn]))
                nc.sync.dma_start(out=r(x2l[:, c0:c0 + cn]),
                                  in_=r(x2l_d[:, c0:c0 + cn]))
            nc.scalar.dma_start(out=r(cpack), in_=r(cpack_d))
            nc.scalar.dma_start(out=selpack, in_=selpack_d)
            nc.vector.memset(ones128, 1.0)
            nc.vector.tensor_copy(id128b, cpack[:, CP_ID128:CP_ID128 + C2])
            nc.vector.tensor_copy(id64b, cpack[:, CP_ID64B:CP_ID64B + C])
            wloRT = cpack[0:C, CP_WLORT:CP_WLORT + C]
            wroRT = cpack[0:C, CP_WRORT:CP_WRORT + C]
            wloLT = cpack[0:C, CP_WLOLT:CP_WLOLT + C]
            wroLT = cpack[0:C, CP_WROLT:CP_WROLT + C]
            vlr = cpack[:, CP_VLR:CP_VLR + 1]
            bL = cpack[0:C, CP_BLR:CP_BLR + 1]
            bR = cpack[C:C2, CP_BLR:CP_BLR + 1]
            bLo = cpack[0:C, CP_BLRO:CP_BLRO + 1]
            bRo0 = cpack[0:C, CP_BRO0:CP_BRO0 + 1]
            gb1 = cpack[:, CP_GB1:CP_GB1 + 1]
            gb2 = cpack[:, CP_GB2:CP_GB2 + 1]

            with contextlib.ExitStack() as ph2_psum:
                p12p = ph2_psum.enter_context(
                    tc.tile_pool(name="p12p", bufs=1, space="PSUM"))
                affp = ph2_psum.enter_context(
                    tc.tile_pool(name="affp", bufs=1, space="PSUM"))
                P12 = p12p.tile([C2, HW], F32)  # 5 banks, lives phase 1+2
                ring = affp.tile([C2, 1536], F32, tag="ring", name="aff_ring")

                # ---- phase 1: convs + Y-strip builds ----
                # Y_t pair = [(W_r@XL_strip).T | (W_r@XR_strip).T] lands in
                # P12 scratch cols 128t:128t+128, one cast-copy to bf16 Wc.
                # GPSIMD can't read PSUM, so copies go 2:1 on DVE:ACT.
                def emit_y(t):
                    ysl = slice(128 * t, 128 * t + 64)
                    nc.tensor.matmul(P12[:, ysl],
                                     r(XL[:, 128 * t:128 * t + 128]),
                                     r(wloRT), start=True, stop=True)
                    ysr = slice(128 * t + 64, 128 * t + 128)
                    nc.tensor.matmul(P12[:, ysr],
                                     r(XR[:, 128 * t:128 * t + 128]),
                                     r(wroRT), start=True, stop=True)
                    wsl = slice(128 * t, 128 * t + 128)
                    if t % 3 == 2:
                        nc.scalar.copy(Wc[:, wsl], P12[:, wsl])
                    else:
                        nc.vector.tensor_copy(Wc[:, wsl], P12[:, wsl])

                for j, (c0, cn) in enumerate(CH_2304):
                    rsl = (j % 3) * 512
                    nc.tensor.matmul(ring[0:C, rsl:rsl + cn], r(wlrT[:, C:C2]),
                                     r(x2r[:, c0:c0 + cn]), start=True,
                                     stop=True)
                    nc.vector.tensor_scalar_add(r(XR[:, c0:c0 + cn]),
                                                ring[0:C, rsl:rsl + cn], bR)
                    nc.tensor.matmul(P12[0:C, c0:c0 + cn], r(wlrT[:, 0:C]),
                                     r(x2l[:, c0:c0 + cn]), start=True,
                                     stop=True)
                    nc.scalar.activation(r(XL[:, c0:c0 + cn]),
                                         P12[0:C, c0:c0 + cn],
                                         AF.Identity, bias=bL, scale=1.0)
                    if j > 0:
                        for t in range(4 * (j - 1), 4 * j):
                            emit_y(t)
                for t in range(4 * (len(CH_2304) - 1), NSTRIP):
                    emit_y(t)

                # ---- phase 2: strip loop ----
                # aff -> ring (3 rotating 512-slots); exp -> bf16 E in SBUF
                # (all strips stay resident, so bacc lags aff by 2 strips with
                # no recycling pressure and PE tracks ACT's pace).
                phase = 2
                r2s = {}
                Es = {}

                def emit_bacc(sb):
                    for c0, cn in CH_2304:
                        nc.tensor.matmul(P12[:, c0:c0 + cn],
                                         Wc[:, 128 * sb:128 * sb + 128],
                                         Es[sb][:, c0:c0 + cn],
                                         start=(sb == 0),
                                         stop=(sb == NSTRIP - 1))

                def emit_strip(s, phase):
                    E = epool.tile([C2, HW], BF16, tag="e", name=f"E_{s}")
                    rs = smalls.tile([C2, 4], F32, tag="rs", name=f"rs_{s}")
                    lhs_aff = r(XL[:, 128 * s:128 * s + 128])
                    pieces = [(p0, pn, (phase + i) % 3)
                              for i, (p0, pn) in enumerate(CH_2304)]
                    groups = []
                    for p0, pn, sl in pieces:
                        if groups and groups[-1][2] + groups[-1][1] == sl * 512 \
                                and groups[-1][1] + pn <= 1536:
                            groups[-1][1] += pn
                        else:
                            groups.append([p0, pn, sl * 512])
                    gidx = 0
                    done = 0
                    for p0, pn, sl in pieces:
                        nc.tensor.matmul(ring[:, sl * 512:sl * 512 + pn],
                                         lhs_aff, r(XR[:, p0:p0 + pn]),
                                         start=True, stop=True)
                        done += pn
                        while gidx < len(groups) and \
                                groups[gidx][0] + groups[gidx][1] <= done:
                            m0, mn, r0 = groups[gidx]
                            nc.scalar.activation(E[:, m0:m0 + mn],
                                                 ring[:, r0:r0 + mn], AF.Exp,
                                                 accum_out=rs[:, gidx:gidx + 1])
                            gidx += 1
                    rowsum = smalls.tile([C2, 1], F32, tag="rowsum",
                                         name=f"rowsum_{s}")
                    r2 = smalls.tile([C2, 1], F32, tag="r2", name=f"r2_{s}",
                                     bufs=4)
                    nc.vector.tensor_reduce(rowsum, rs[:, 0:len(groups)],
                                            axis=mybir.AxisListType.X,
                                            op=mybir.AluOpType.add)
                    nc.vector.reciprocal(r2, rowsum)
                    r2s[s] = r2
                    wright = Wc[:, 128 * s + 64:128 * s + 128]
                    nc.vector.tensor_scalar_mul(wright, wright, r2)
                    # colsum accumulate: DVE takes [0:1536), Pool the rest
                    if s == 0:
                        nc.vector.tensor_copy(csum_a, E[:, 0:CSPLIT])
                        nc.gpsimd.tensor_copy(csum_b, E[:, CSPLIT:HW])
                    else:
                        nc.vector.tensor_add(csum_a, csum_a, E[:, 0:CSPLIT])
                        nc.gpsimd.tensor_add(csum_b, csum_b, E[:, CSPLIT:HW])
                    return E

                for s in range(NSTRIP):
                    Es[s] = emit_strip(s, phase)
                    phase = (phase + len(CH_2304)) % 3
                    if s >= 2:
                        emit_bacc(s - 2)
                emit_bacc(NSTRIP - 2)
                emit_bacc(NSTRIP - 1)

                # drain P12 to SBUF (GPSIMD can't read PSUM: DVE/ACT halves)
                nc.vector.tensor_copy(P12sb[:, 0:1152], P12[:, 0:1152])
                nc.scalar.copy(P12sb[:, 1152:2304], P12[:, 1152:2304])

            # ---- phase 3 ----
            with tc.tile_pool(name="ph3p", bufs=1, space="PSUM") as ph3, \
                    tc.tile_pool(name="ph3r", bufs=2, space="PSUM") as ph3r:
                colg = ph3.tile([C2, 3 * NSTRIP], F32, name="colg",
                                padded_shape=[C2, 512])
                colT = colg[:, 0:NSTRIP]
                g12pT = colg[:, NSTRIP:3 * NSTRIP]
                for q in range(NSTRIP):
                    blk = slice(128 * q, 128 * q + 128)
                    if 128 * q + 128 <= CSPLIT:
                        src = csum_a[:, blk]
                    else:
                        src = csum_b[:, 128 * q - CSPLIT:128 * q - CSPLIT + 128]
                    nc.tensor.matmul(colT[:, q:q + 1], src, ones128,
                                     start=True, stop=True)
                r1T = ph3sb.tile([C2, NSTRIP], F32, name="r1T")
                nc.vector.reciprocal(r1T, colT)
                for q in range(NSTRIP):
                    blk = slice(128 * q, 128 * q + 128)
                    nc.tensor.matmul(g12pT[:, q:q + 1], P12sb[0:C, blk],
                                     vlr[0:C], start=True, stop=True)
                    nc.tensor.matmul(g12pT[:, NSTRIP + q:NSTRIP + q + 1],
                                     P12sb[C:C2, blk], vlr[C:C2],
                                     start=True, stop=True)
                g1preT = ph3sb.tile([C2, NSTRIP], F32, name="g1preT")
                nc.vector.tensor_mul(g1preT, g12pT[:, 0:NSTRIP], r1T)
                g1T = ph3sb.tile([C2, NSTRIP], F32, name="g1T")
                nc.scalar.activation(g1T, g1preT, AF.Sigmoid, bias=gb1,
                                     scale=1.0)
                s12T = ph3sb.tile([C2, 2 * NSTRIP], BF16, name="s12T")
                nc.vector.tensor_mul(s12T[:, 0:NSTRIP], g1T, r1T)
                nc.scalar.activation(s12T[:, NSTRIP:2 * NSTRIP],
                                     g12pT[:, NSTRIP:2 * NSTRIP], AF.Sigmoid,
                                     bias=gb2, scale=1.0)
                # transpose to [36, 128] rows (q-major) so S12 can be built by
                # per-block selector matmuls against SBUF data
                sT_ps = ph3.tile([2 * NSTRIP, C2], BF16, name="sT_ps")
                nc.tensor.transpose(sT_ps, s12T, id128b)
                sT_sb = ph3sb.tile([2 * NSTRIP, C2], BF16, name="sT_sb")
                nc.vector.tensor_copy(sT_sb, sT_ps)

                for ci, (c0, cn) in enumerate(CH_2304):
                    S12 = ph3r.tile([C2, cn], F32, tag="S12", name=f"S12_{ci}",
                                    padded_shape=[C2, 512])
                    for b in range(cn // 128):
                        m0 = c0 + 128 * b
                        nc.tensor.matmul(S12[:, 128 * b:128 * b + 128],
                                         selpack[:, m0:m0 + 128], sT_sb,
                                         start=True, stop=True)
                    t12 = ph3sb.tile([C2, cn], BF16, tag="t12",
                                     name=f"t12_{ci}", padded_shape=[C2, 512])
                    nc.vector.tensor_mul(t12, P12sb[:, c0:c0 + cn], S12)
                    OL = ph3r.tile([C, cn], F32, tag="OL", name=f"OL_{ci}",
                                   padded_shape=[C, 512])
                    nc.tensor.matmul(OL, r(wloLT), r(XL[:, c0:c0 + cn]),
                                     start=True, stop=False)
                    nc.tensor.matmul(OL, id64b[0:C], t12[0:C, :],
                                     start=False, stop=True)
                    nc.scalar.activation(outLR[0:C, c0:c0 + cn], OL,
                                         AF.Identity, bias=bLo, scale=1.0)
                    OR_ = ph3r.tile([C, cn], F32, tag="OR", name=f"OR_{ci}",
                                    padded_shape=[C, 512])
                    nc.tensor.matmul(OR_, r(wroLT), r(XR[:, c0:c0 + cn]),
                                     start=True, stop=False)
                    nc.tensor.matmul(OR_, id64b[C:C2], t12[C:C2, :],
                                     start=False, stop=True)
                    nc.scalar.activation(outLR[C:C2, c0:c0 + cn], OR_,
                                         AF.Identity, bias=bRo0, scale=1.0)
                    if ci == 1:
                        nc.sync.dma_start(out=out_l_d[:, 0:1024],
                                          in_=outLR[0:C, 0:1024])
                        nc.scalar.dma_start(out=out_r_d[:, 0:1024],
                                            in_=outLR[C:C2, 0:1024])
                nc.sync.dma_start(out=out_l_d[:, 1024:HW],
                                  in_=outLR[0:C, 1024:HW])
                nc.scalar.dma_start(out=out_r_d[:, 1024:HW],
                                    in_=outLR[C:C2, 1024:HW])

    nc.compile()
    return nc


_NC_CACHE = {}


def _get_nc():
    if "nc" not in _NC_CACHE:
        _NC_CACHE["nc"] = build_nc()
    return _NC_CACHE["nc"]


def _prep_shared(concaL_w, concaL_b, concaR_w, concaR_b,
                 gateL_w, gateL_b, gateR_w, gateR_b,
                 concaLo_w, concaLo_b, concaRo_w, concaRo_b):
    f = np.float32
    wloR = np.asarray(concaLo_w)[:, C:].astype(np.float64)
    wroR = np.asarray(concaRo_w)[:, C:].astype(np.float64)
    vL = np.linalg.solve(wloR.T, np.asarray(gateL_w).astype(np.float64).reshape(C))
    vR = np.linalg.solve(wroR.T, np.asarray(gateR_w).astype(np.float64).reshape(C))
    wlrT = np.concatenate([np.asarray(concaL_w).T, np.asarray(concaR_w).T],
                          axis=1)

    cpack = np.zeros((C2, NCPACK), dtype=f)
    cpack[0:C, CP_WLORT:CP_WLORT + C] = wloR.T
    cpack[0:C, CP_WRORT:CP_WRORT + C] = wroR.T
    cpack[0:C, CP_WLOLT:CP_WLOLT + C] = np.asarray(concaLo_w)[:, :C].T
    cpack[0:C, CP_WROLT:CP_WROLT + C] = np.asarray(concaRo_w)[:, :C].T
    cpack[:, CP_VLR] = np.concatenate([vL, vR])
    cpack[0:C, CP_BLR] = np.asarray(concaL_b).reshape(C)
    cpack[C:C2, CP_BLR] = np.asarray(concaR_b).reshape(C)
    cpack[0:C, CP_BLRO] = np.asarray(concaLo_b).reshape(C)
    cpack[C:C2, CP_BLRO] = np.asarray(concaRo_b).reshape(C)
    cpack[0:C, CP_BRO0] = np.asarray(concaRo_b).reshape(C)
    cpack[:, CP_GB1] = np.asarray(gateL_b).reshape(())
    cpack[:, CP_GB2] = np.asarray(gateR_b).reshape(())
    eye = np.eye(C, dtype=f)
    cpack[0:C, CP_ID64B:CP_ID64B + C] = eye
    cpack[C:C2, CP_ID64B:CP_ID64B + C] = eye
    cpack[:, CP_ID128:CP_ID128 + C2] = np.eye(C2, dtype=f)
    import ml_dtypes
    # selpack[k, 128q+c] = 1 iff (c<64 and k==q) or (c>=64 and k==18+q):
    # S12 block q = selpack[:, blk].T @ sT_sb broadcasts s1/s2 rows of sT
    # over the channel halves of the output.
    selpack = np.zeros((2 * NSTRIP, HW), dtype=np.float32)
    for q in range(NSTRIP):
        selpack[q, 128 * q:128 * q + 64] = 1.0
        selpack[NSTRIP + q, 128 * q + 64:128 * q + 128] = 1.0
    return {
        "wlrT": np.ascontiguousarray(wlrT, dtype=f),
        "cpack": np.ascontiguousarray(cpack, dtype=f),
        "selpack": np.ascontiguousarray(selpack.astype(ml_dtypes.bfloat16)),
    }


def kernel(xlh, xll, xrh, xrl,
           concaL_w, concaL_b, concaR_w, concaR_b,
           gateL_w, gateL_b, gateR_w, gateR_b,
           concaLo_w, concaLo_b, concaRo_w, concaRo_b,
           _return_results=False):
    nc = _get_nc()
    shared = _prep_shared(concaL_w, concaL_b, concaR_w, concaR_b,
                          gateL_w, gateL_b, gateR_w, gateR_b,
                          concaLo_w, concaLo_b, concaRo_w, concaRo_b)
    xlh = np.asarray(xlh, dtype=np.float32)
    xll = np.asarray(xll, dtype=np.float32)
    xrh = np.asarray(xrh, dtype=np.float32)
    xrl = np.asarray(xrl, dtype=np.float32)

    in_maps = []
    for c in range(B):
        x2l = np.concatenate([xlh[c].reshape(C, HW), xll[c].reshape(C, HW)], axis=0)
        x2r = np.concatenate([xrh[c].reshape(C, HW), xrl[c].reshape(C, HW)], axis=0)
        m = dict(shared)
        m["x2l"] = np.ascontiguousarray(x2l)
        m["x2r"] = np.ascontiguousarray(x2r)
        in_maps.append(m)

    # The first execution of a freshly compiled NEFF occasionally hits a
    # transient NRT_EXEC_UNIT_UNRECOVERABLE on this axon setup; an immediate
    # re-dispatch of the same executable has always succeeded, so retry.
    res = None
    for attempt in range(3):
        try:
            res = run_bass_kernel_spmd(nc, in_maps, list(range(B)))
            break
        except Exception:
            if attempt == 2:
                raise
            import time as _time
            _time.sleep(2.0)
    out_L = np.stack([res.results[c]["out_l"].reshape(C, H, W) for c in range(B)])
    out_R = np.stack([res.results[c]["out_r"].reshape(C, H, W) for c in range(B)])
    if _return_results:
        return (out_L, out_R), res
    return (out_L, out_R)
